# revision 18
# baseline (speedup 1.0000x reference)
"""Trainium2 Bass kernel for nn_CRF_16389595202091 (v2 design).

CRF: dense projection [B,T,D]x[D,U] -> potentials, Viterbi decode, return
(pot, onehot(tags)).  Data-parallel over batch: 8 cores x 8 batches.
Shapes hardcoded: B=64, T=1024, D=1024, U=64, mask all-ones.

v2 forward loop (per core, per step) -- no per-step broadcast matmuls:
  state lives as mstate [128, 4]: partition p=(g,i) (g=p//64, i=p%64),
  col h in 0..3; batch = M_par[g][h] with parity-alternating mapping
  (an involution pi: M1(g',h') = 4*(h'%2) + 2*(h'//2) + g').
  1. DVE TT: scores[p=(g,i), (h,j)] = mstate_bc + transbc (const)  [128,256]
  2. PE: 2x transpose halves -> ps_tr [p=(hh,j), (H,g,i)] PSUM [128,256]
  3. DVE reduce max over i -> maxfix' [128, 4] (cols c'=(H,g))
  4. DVE TT tiny: mstate' = maxfix' + potT2 col (parity slot view)
  5. bp (deferred 1 step): mfneg = -HUGE*maxfix (DVE ts);
     4x ACT: z1_c = Abs(HUGE*ps_tr_c + mfneg_c)  (exact: 0 at argmax)
     4x DVE ttr: bphist[:,s,c] = max_i(wrev - z1_c)  = 63 - argmax_i
     (tie-safe: exact-0 only at fp32-max entries; lowest-i wins like jnp.argmax)
  Final pseudo-step s=T with zero trans term yields argmax of the last state.

Then: bulk bp fix (63-x), parity-split PE transposes -> bpnat8 [8,...] u8,
1024-step backward chase (1 DVE stt per step, tags accumulated into taghist),
bulk onehot via is_equal TT on 128 partitions, DMA out.
"""

import os
import numpy as np

B, T, D, U = 64, 1024, 1024, 64
NB = B // 8          # batches per core
NTOK = NB * T        # tokens per core
FWDSTEPS = int(os.environ.get("CRF_FWDSTEPS", str(T)))
BPMODE = os.environ.get("CRF_BPMODE", "red")  # "ttr" | "red"
HUGE = float(2.0 ** 30)

_cached = {}

# batch mapping tables: M_par[g][h] = batch at (partition-half g, col h)
M0 = [[4 * g + h for h in range(4)] for g in range(2)]
M1 = [[4 * (h % 2) + 2 * (h // 2) + g for h in range(4)] for g in range(2)]
MS = [M0, M1]
# potT2 slot layout [128, 8, 1024]: half g slots 0..3 = M0[g], 4..7 = M1[g]
SLOTB = [M0[0] + M1[0], M0[1] + M1[1]]


def _build_nc():
    import concourse.bass as bass
    import concourse.bacc as bacc
    import concourse.mybir as mybir
    from concourse.tile import TileContext

    f32 = mybir.dt.float32
    u8 = mybir.dt.uint8
    AX = mybir.AxisListType.X
    OP = mybir.AluOpType
    AF = mybir.ActivationFunctionType

    nc = bacc.Bacc("TRN2", target_bir_lowering=False, debug=False, num_devices=8)

    x_d = nc.dram_tensor("x", [D, NTOK], f32, kind="ExternalInput")
    w_d = nc.dram_tensor("w", [D, U], f32, kind="ExternalInput")
    bcol_d = nc.dram_tensor("bcol", [U, 1], f32, kind="ExternalInput")
    lr_d = nc.dram_tensor("lr", [U, 2], f32, kind="ExternalInput")
    i128_d = nc.dram_tensor("i128", [128, 128], f32, kind="ExternalInput")
    transbc_d = nc.dram_tensor("transbc", [128, 256], f32, kind="ExternalInput")
    wrev_d = nc.dram_tensor("wrev", [128, 256], f32, kind="ExternalInput")
    iota8_d = nc.dram_tensor("iota8", [8, 64], f32, kind="ExternalInput")
    iotaU_d = nc.dram_tensor("iotaU", [128, 64], f32, kind="ExternalInput")

    pot_d = nc.dram_tensor("pot_out", [NB, T, U], f32, kind="ExternalOutput")
    oh_d = nc.dram_tensor("oh_out", [NB, T, U], f32, kind="ExternalOutput")

    with TileContext(nc) as tc:
        with tc.tile_pool(name="const", bufs=1) as cpool:
            i128 = cpool.tile([128, 128], f32)
            nc.sync.dma_start(i128[:], i128_d[:])
            i64dup = cpool.tile([128, 64], f32)
            nc.sync.dma_start(i64dup[0:64, :], i128_d[0:64, 0:64])
            nc.sync.dma_start(i64dup[64:128, :], i128_d[0:64, 0:64])
            transbc = cpool.tile([128, 4, 64], f32)
            nc.sync.dma_start(transbc[:], transbc_d[:].rearrange(
                "p (a b) -> p a b", a=4))
            wrev = cpool.tile([128, 4, 64], f32)
            nc.sync.dma_start(wrev[:], wrev_d[:].rearrange(
                "p (a b) -> p a b", a=4))
            iota8 = cpool.tile([8, 64], f32)
            nc.sync.dma_start(iota8[:], iota8_d[:])
            iotaU = cpool.tile([128, 64], f32)
            nc.sync.dma_start(iotaU[:], iotaU_d[:])
            bcol = cpool.tile([64, 1], f32)
            nc.sync.dma_start(bcol[:], bcol_d[:])
            lrsb = cpool.tile([64, 2], f32)
            nc.sync.dma_start(lrsb[:], lr_d[:])
            wsb = cpool.tile([128, 8, 64], f32)
            nc.sync.dma_start(wsb[:], w_d[:].rearrange("(c p) u -> p c u", p=128))
            zero4 = cpool.tile([128, 4, 64], f32)
            nc.vector.memset(zero4[:], 0.0)

            midp = tc.alloc_tile_pool(name="mid", bufs=1)
            potT2 = midp.tile([128, 8, 1024], f32)
            bphist = midp.tile([128, T + 2, 4], f32)
            nc.vector.memset(bphist[:], 0.0)

            # ================= Stage A: projection =================
            with tc.tile_pool(name="xin", bufs=3) as xinp, \
                 tc.tile_pool(name="pp", bufs=3) as ppp, \
                 tc.tile_pool(name="po", bufs=4) as pop, \
                 tc.tile_pool(name="pspot", bufs=2, space="PSUM") as pspotp, \
                 tc.tile_pool(name="ptp", bufs=4, space="PSUM") as ptpp:
                for g4 in range(NTOK // 512):
                    b, half = g4 // 2, g4 % 2
                    # x is host-pre-transposed: [D, NTOK]
                    xtT = xinp.tile([128, 8, 512], f32)
                    nc.sync.dma_start(
                        xtT[:], x_d[:, g4 * 512:(g4 + 1) * 512].rearrange(
                            "(c p) t -> p c t", p=128))
                    pspot = pspotp.tile([64, 512], f32)
                    for dc in range(8):
                        nc.tensor.matmul(pspot[:], lhsT=wsb[:, dc, :],
                                         rhs=xtT[:, dc, :],
                                         start=(dc == 0), stop=(dc == 7))
                    # bias add PSUM->SBUF
                    pp = ppp.tile([64, 512], f32)
                    nc.vector.tensor_scalar(pp[:], pspot[:], bcol[:], None, OP.add)
                    # boundary energies (mask all ones): t=0 / t=T-1
                    if half == 0:
                        nc.vector.tensor_scalar(pp[:, 0:1], pp[:, 0:1],
                                                lrsb[:, 0:1], None, OP.add)
                    else:
                        nc.vector.tensor_scalar(pp[:, 511:512], pp[:, 511:512],
                                                lrsb[:, 1:2], None, OP.add)
                    # two slot copies into potT2
                    slots = [(g, l) for g in range(2) for l in range(8)
                             if SLOTB[g][l] == b]
                    for k, (g, l) in enumerate(slots):
                        dst = potT2[g * 64:(g + 1) * 64, l,
                                    half * 512:(half + 1) * 512]
                        if k == 0:
                            nc.vector.tensor_copy(dst, pp[:])
                        else:
                            nc.scalar.copy(dst, pp[:])
                    # pot output: transpose [64u,512t] -> 4x [128t, 64u]
                    for q in range(4):
                        ptp = ptpp.tile([128, 64], f32)
                        nc.tensor.transpose(ptp[:], pp[:, q * 128:(q + 1) * 128],
                                            i128[0:64, 0:64])
                        po = pop.tile([128, 64], f32)
                        nc.scalar.copy(po[:], ptp[:])
                        t0 = half * 512 + q * 128
                        nc.sync.dma_start(pot_d[b, t0:t0 + 128, :], po[:])

            # ================= Stage C: forward =================
            with tc.tile_pool(name="sc", bufs=3) as scp, \
                 tc.tile_pool(name="mx", bufs=8) as mxp, \
                 tc.tile_pool(name="zz", bufs=4) as zzp, \
                 tc.tile_pool(name="pstr2", bufs=3, space="PSUM") as ps2p:

                pending = []  # (z1, s) awaiting z2/red2

                def emit_z1(ps_tr, maxfix, s):
                    # mfneg + 4x ACT Abs -> z1 (= HUGE*|score - max|)
                    # demoted priority: must not win ready-ties against the
                    # critical red->mstate->scores chain on the DVE.
                    with tc.high_priority(offset=-60):
                        mfneg = mxp.tile([128, 4], f32, name="mfneg")
                        nc.vector.tensor_scalar(mfneg[:], maxfix[:], -HUGE,
                                                None, OP.mult)
                        z1 = zzp.tile([128, 4, 64], f32, name="z1")
                        v = ps_tr[:].rearrange("p (a b) -> p a b", a=4)
                        for c in range(4):
                            nc.scalar.activation(z1[:, c, :], v[:, c, :],
                                                 AF.Abs,
                                                 bias=mfneg[:, c:c + 1],
                                                 scale=HUGE)
                    return z1

                def emit_bp(z1, s):
                    with tc.high_priority(offset=-60):
                        # z2 on GPSIMD (idle engine, SBUF-only operands) so
                        # only red2 competes with the critical DVE chain.
                        nc.gpsimd.tensor_tensor(out=z1[:], in0=wrev[:],
                                                in1=z1[:], op=OP.subtract)
                        nc.vector.tensor_reduce(bphist[:, s, :], z1[:],
                                                axis=AX, op=OP.max)

                # software-pipelined: scores for step s+1 emitted right after
                # mstate_s; bp DVE work deferred 2 steps into the tp window.
                scores = scp.tile([128, 4, 64], f32, name="scores")
                nc.vector.tensor_tensor(
                    out=scores[:],
                    in0=potT2[:, 0:4, 0:1].broadcast_to([128, 4, 64]),
                    in1=transbc[:], op=OP.add)
                for s in range(1, FWDSTEPS + 1):
                    ps_tr = ps2p.tile([128, 256], f32, name="ps_tr")
                    sflat = scores[:].rearrange("p a b -> p (a b)")
                    nc.tensor.transpose(ps_tr[:, 0:128], sflat[:, 0:128],
                                        i128[:])
                    nc.tensor.transpose(ps_tr[:, 128:256], sflat[:, 128:256],
                                        i128[:])
                    maxfix = mxp.tile([128, 4], f32, name="maxfix")
                    nc.vector.tensor_reduce(
                        maxfix[:], ps_tr[:].rearrange("p (a b) -> p a b", a=4),
                        axis=AX, op=OP.max)
                    if s < FWDSTEPS:
                        mstate = mxp.tile([128, 4], f32, name="mstate")
                        off = 4 * (s % 2)
                        nc.vector.tensor_tensor(
                            out=mstate[:], in0=maxfix[:],
                            in1=potT2[:, off:off + 4, s], op=OP.add)
                        scores = scp.tile([128, 4, 64], f32, name="scores")
                        nc.vector.tensor_tensor(
                            out=scores[:],
                            in0=mstate[:].unsqueeze(2).broadcast_to(
                                [128, 4, 64]),
                            in1=transbc[:] if s + 1 < T else zero4[:],
                            op=OP.add)
                    z1 = emit_z1(ps_tr, maxfix, s)
                    pending.append((z1, s))
                    if len(pending) > 2:
                        emit_bp(*pending.pop(0))
                for args in pending:
                    emit_bp(*args)

            # ================= Stage D: bp fix + relayout =================
            dpool = tc.alloc_tile_pool(name="dpool", bufs=1, side="right")
            # bphist holds (63 - bp); fix in place
            bpv = bphist[:].rearrange("p t a -> p (t a)")
            nc.vector.tensor_scalar(bpv, bpv, -1.0, 63.0, OP.mult, OP.add)

            # bpT1 [64 r, 8 b, 2 P, 8 blk, 64 j] u8 ; s = 2*(blk*64+r) + P
            bpT1 = dpool.tile([64, 8, 2, 8, 64], u8)
            bph4 = bphist[:, 0:T, :].rearrange("p (t par) a -> p t par a",
                                               par=2)
            with tc.tile_pool(name="psd", bufs=2, space="PSUM") as psdp:
                for P in range(2):
                    for blk in range(8):
                        psd = psdp.tile([64, 512], f32)
                        for g in range(2):
                            for h in range(4):
                                bb = MS[P][g][h]
                                src = bph4[g * 64:(g + 1) * 64,
                                           blk * 64:(blk + 1) * 64, P, h]
                                dst = psd[:, bb * 64:(bb + 1) * 64]
                                if g == 0:
                                    nc.tensor.transpose(dst, src,
                                                        i64dup[0:64, :])
                                else:
                                    nc.tensor.matmul(dst, lhsT=src,
                                                     rhs=i64dup[64:128, :],
                                                     start=True, stop=True,
                                                     skip_group_check=True)
                        dst = bpT1[:, :, P, blk, :]
                        if blk % 2 == 0:
                            nc.vector.tensor_copy(
                                dst, psd[:].rearrange("r (b x) -> r b x", b=8))
                        else:
                            nc.scalar.copy(
                                dst, psd[:].rearrange("r (b x) -> r b x", b=8))

            bpnat8 = dpool.tile([8, 64, 2, 8, 64], u8)
            for b in range(8):
                nc.sync.dma_start(bpnat8[b:b + 1], bpT1[:, b, :, :, :])

            # ================= Stage E: backward chase =================
            midp.release()
            with tc.tile_pool(name="chs", bufs=1) as chp:
                taghist = chp.tile([8, 1024], f32)
                ohjunk = chp.tile([8, 64], f32)
                # final tags: bphist[:, 1024]: parity 0 -> M0: b = 4g+h, j=0 row
                for b in range(8):
                    g, h = b // 4, b % 4
                    nc.sync.dma_start(taghist[b:b + 1, 1023:1024],
                                      bphist[g * 64:g * 64 + 1, 1024, h:h + 1])
                for s in range(T - 1, 0, -1):
                    r, blk, P = (s // 2) % 64, (s // 2) // 64, s % 2
                    nc.vector.scalar_tensor_tensor(
                        out=ohjunk[:], in0=iota8[:], scalar=taghist[:, s:s + 1],
                        in1=bpnat8[:, r, P, blk, :],
                        op0=OP.is_equal, op1=OP.mult,
                        accum_out=taghist[:, s - 1:s])
                # bulk onehot: taghist [8,1024] -> tgT [128, 8c, 8b]
                tgT = chp.tile([128, 8, 8], f32)
                ohbig = chp.tile([128, 8, 8, 64], f32)
                with tc.tile_pool(name="pst", bufs=2, space="PSUM") as pstp:
                    for c in range(8):
                        pst = pstp.tile([128, 8], f32)
                        nc.tensor.transpose(
                            pst[:], taghist[:, c * 128:(c + 1) * 128],
                            i128[0:8, 0:8])
                        if c % 2 == 0:
                            nc.vector.tensor_copy(tgT[:, c, :], pst[:])
                        else:
                            nc.scalar.copy(tgT[:, c, :], pst[:])
                nc.vector.tensor_tensor(
                    out=ohbig[:],
                    in0=tgT[:].unsqueeze(3).broadcast_to([128, 8, 8, 64]),
                    in1=iotaU[:].unsqueeze(1).unsqueeze(1).broadcast_to(
                        [128, 8, 8, 64]),
                    op=OP.is_equal)
                for b in range(8):
                    nc.sync.dma_start(
                        oh_d[b].rearrange("(c r) u -> r c u", c=8),
                        ohbig[:, :, b, :])
            dpool.release()

    nc.finalize()
    return nc


def _consts():
    i128 = np.eye(128, dtype=np.float32)
    iota8 = np.tile(np.arange(64, dtype=np.float32)[None, :], (8, 1))
    iotaU = np.tile(np.arange(64, dtype=np.float32)[None, :], (128, 1))
    wrev = np.tile((63 - np.arange(64, dtype=np.float32))[None, None, :],
                   (128, 4, 1)).reshape(128, 256)
    return i128, iota8, iotaU, wrev


def kernel(inputs, mask, W, b, trans, left_b, right_b):
    from concourse.bass_utils import run_bass_kernel_spmd

    if "nc" not in _cached:
        _cached["nc"] = _build_nc()
    nc = _cached["nc"]

    inputs = np.ascontiguousarray(np.asarray(inputs, np.float32))
    W = np.ascontiguousarray(np.asarray(W, np.float32))
    bvec = np.asarray(b, np.float32).reshape(U, 1)
    trans = np.asarray(trans, np.float32)
    lr = np.stack([np.asarray(left_b, np.float32),
                   np.asarray(right_b, np.float32)], axis=1)
    i128, iota8, iotaU, wrev = _consts()
    # transbc [128, 256]: p=(g,i), cols (h,j): trans[i, j]
    i_of_p = np.arange(128) % 64
    transbc = np.tile(trans[i_of_p][:, None, :], (1, 4, 1)).reshape(128, 256)

    in_maps = []
    for c in range(8):
        shard = inputs[c * NB:(c + 1) * NB].reshape(NTOK, D)
        in_maps.append({
            "x": np.ascontiguousarray(shard.T),
            "w": W, "bcol": bvec,
            "lr": np.ascontiguousarray(lr),
            "i128": i128,
            "transbc": np.ascontiguousarray(transbc.astype(np.float32)),
            "wrev": np.ascontiguousarray(wrev),
            "iota8": np.ascontiguousarray(iota8),
            "iotaU": np.ascontiguousarray(iotaU),
        })

    trace = bool(int(os.environ.get("CRF_TRACE", "0")))
    res = run_bass_kernel_spmd(nc, in_maps, core_ids=list(range(8)),
                               trace=trace)
    if trace:
        print("HW exec time:", res.exec_time_ns, "ns")
        print("mean exec:", res.mean_exec_time_ns, "trace:",
              res.instructions_and_trace[1] if res.instructions_and_trace else None)
    pot = np.concatenate([r["pot_out"] for r in res.results], axis=0)
    onehot = np.concatenate([r["oh_out"] for r in res.results], axis=0)
    return pot, onehot
